# revision 10
# baseline (speedup 1.0000x reference)
"""Trainium2 Bass kernel for the CapsuleNetwork routing problem.

Problem (per reference):
  B, L, D, K = 1024, 200, 64, 4 ; E = K*D = 256
  hat[b,l,e] = sum_d seq[b,l,d] * W[l,e,d]          (einsum, PE)
  3 rounds of dynamic routing over interests K (softmax over K per (b,l)),
  cap = squash(w @ hat), cw += hat . cap            (DVE/ACT)
  output cap[:, :, 0, :]  -> [B, K, D]

Sharding: pure data-parallel over batch across 8 NeuronCores (128 rows each);
weights replicated. All layout transforms (transposes for the d'-contraction
matmuls) are done host-side so the device sees clean burst DMAs.

Restructured routing algebra (validated vs reference to ~3e-7):
  cw layout [B, L, K];   w = exp(cw) / sum_k exp(cw)
  capRaw[b,k,:] = sum_l w[b,l,k] hat[b,l,k,:]
  n = |capRaw|^2 ; s = n/(1+n)/sqrt(n+1e-9)
  cw += s[b,k] * (hat . capRaw)    (squash scale folded into the cw update)
  final out = s * capRaw
"""

import os
import sys

import numpy as np

for _p in ("/opt/trn_rl_repo", "/root/.axon_site/_ro/trn_rl_repo"):
    if os.path.isdir(_p) and _p not in sys.path:
        sys.path.insert(0, _p)

B, L, D, K = 1024, 200, 64, 4
E = K * D
NCORES = 8
BS = B // NCORES  # 128 batch rows per core
M = L // 2        # l-pairs: partition p = (l%2)*64 + d'
F32 = None        # set after imports


def build_nc():
    """Build the Bass program for one core (SPMD; all cores run the same NEFF)."""
    import concourse.bass as bass
    import concourse.tile as tile
    from concourse import bacc, mybir

    f32 = mybir.dt.float32
    AF = mybir.ActivationFunctionType
    OP = mybir.AluOpType

    nc = bacc.Bacc(trn_type="TRN2", target_bir_lowering=False, debug=False)
    seqT_d = nc.dram_tensor("seqT", [128, M, BS], f32, kind="ExternalInput")
    wT_d = nc.dram_tensor("wT", [128, M, E], f32, kind="ExternalInput")
    cw_d = nc.dram_tensor("cw", [BS, L, K], f32, kind="ExternalInput")
    out_d = nc.dram_tensor("out", [BS, E], f32, kind="ExternalOutput")

    NACC = 4  # rotating accumulator slots per k

    with tile.TileContext(nc) as tc:
        with (
            tc.tile_pool(name="consts", bufs=1) as consts,
            tc.tile_pool(name="hatps", bufs=4, space="PSUM") as psum,
            tc.tile_pool(name="hats", bufs=8) as hats,
            tc.tile_pool(name="scr", bufs=4) as scr,
        ):
            seqT = consts.tile([128, M, BS], f32, name="seqT_sb")
            wT = consts.tile([128, M, E], f32, name="wT_sb")
            cw = consts.tile([BS, L, K], f32, name="cw_sb")
            w = consts.tile([BS, L, K], f32, name="w_sb")
            zsum = consts.tile([BS, L], f32, name="zsum")
            zinv = consts.tile([BS, L], f32, name="zinv")
            deltaB = consts.tile([BS, L, K], f32, name="deltaB")
            capAcc = consts.tile([BS, K, NACC, D], f32, name="capAcc")
            capRaw = consts.tile([BS, K, D], f32, name="capRaw")
            capOut = consts.tile([BS, E], f32, name="capOut")
            nvec = consts.tile([BS, K], f32, name="nvec")
            lnt = consts.tile([BS, K], f32, name="lnt")
            rt = consts.tile([BS, K], f32, name="rt")
            np1 = consts.tile([BS, K], f32, name="np1")
            den = consts.tile([BS, K], f32, name="den")
            dinv = consts.tile([BS, K], f32, name="dinv")
            svec = consts.tile([BS, K], f32, name="svec")
            epsB = consts.tile([BS, 1], f32, name="epsB")
            nc.vector.memset(epsB[:], 1e-9)

            # "Bridge" tiles: tiny DVE copies that absorb a cross-engine wait
            # so no DVE instruction needs 2 semaphore waits (the ISA structs
            # fit only one). Each bridge gets its own tile (avoids WAW deps).
            bridge_tiles = [
                consts.tile([BS, 1], f32, name=f"bridge{i}") for i in range(16)
            ]
            bridge_n = [0]

            def bridge(src_ap):
                bt = bridge_tiles[bridge_n[0]]
                bridge_n[0] += 1
                nc.vector.tensor_copy(out=bt[:], in_=src_ap)

            nc.sync.dma_start(out=seqT[:], in_=seqT_d[:])
            nc.sync.dma_start(out=wT[:], in_=wT_d[:])
            nc.sync.dma_start(out=cw[:], in_=cw_d[:])

            # The PE LDWEIGHTS struct fits only one semaphore wait; a matmul
            # that waits on both input DMAs fails walrus codegen. Absorb the
            # seqT DMA wait with a 1x1 dummy matmul so every real matmul
            # carries at most the wT wait.
            with tc.tile_pool(name="dummyp", bufs=1, space="PSUM") as dummyp:
                dps = dummyp.tile([1, 1], f32, name="dps")
                nc.tensor.matmul(
                    dps[:],
                    lhsT=seqT[0:64, 0, 0:1],
                    rhs=seqT[0:64, 0, 0:1],
                    start=True,
                    stop=True,
                )

            def softmax():
                # w = softmax over k of cw (k is innermost, contiguous)
                nc.scalar.activation(out=w[:], in_=cw[:], func=AF.Exp)
                bridge(w[:, 0, 0:1])
                nc.vector.tensor_reduce(
                    out=zsum[:], in_=w[:], axis=mybir.AxisListType.X, op=OP.add
                )
                nc.vector.reciprocal(out=zinv[:], in_=zsum[:])
                for k in range(K):
                    nc.vector.tensor_mul(
                        out=w[:, :, k], in0=w[:, :, k], in1=zinv[:]
                    )

            def stream(consume):
                # recompute hat_l = seqT_l.T @ WT_l for every l, hand the SBUF
                # copy to `consume`
                for l in range(L):
                    par, m = l % 2, l // 2
                    p0 = 64 * par
                    ps = psum.tile([128, E], f32, name="ps", tag="ps")
                    nc.tensor.matmul(
                        ps[:],
                        lhsT=seqT[p0 : p0 + 64, m, :],
                        rhs=wT[p0 : p0 + 64, m, :],
                        start=True,
                        stop=True,
                    )
                    hs = hats.tile([128, E], f32, name="hs", tag="hs")
                    nc.scalar.copy(out=hs[:], in_=ps[:])
                    if l == 0:
                        bridge(hs[:, 0:1])
                    consume(l, hs)

            def capacc_consume(l, hs):
                j = l % NACC
                for k in range(K):
                    nc.vector.scalar_tensor_tensor(
                        out=capAcc[:, k, j, :],
                        in0=hs[:, k * D : (k + 1) * D],
                        scalar=w[:, l, k : k + 1],
                        in1=capAcc[:, k, j, :],
                        op0=OP.mult,
                        op1=OP.add,
                    )

            def delta_consume(l, hs):
                for k in range(K):
                    u = scr.tile([128, D], f32, name="u", tag="u")
                    nc.vector.scalar_tensor_tensor(
                        out=u[:],
                        in0=hs[:, k * D : (k + 1) * D],
                        scalar=1.0,
                        in1=capRaw[:, k, :],
                        op0=OP.mult,
                        op1=OP.mult,
                        accum_out=deltaB[:, l, k : k + 1],
                    )

            def combine_and_squash_scalars():
                # capRaw = sum of the NACC accumulator slots
                nc.vector.tensor_add(
                    out=capAcc[:, :, 0, :],
                    in0=capAcc[:, :, 0, :],
                    in1=capAcc[:, :, 1, :],
                )
                nc.vector.tensor_add(
                    out=capAcc[:, :, 2, :],
                    in0=capAcc[:, :, 2, :],
                    in1=capAcc[:, :, 3, :],
                )
                nc.vector.tensor_add(
                    out=capRaw[:], in0=capAcc[:, :, 0, :], in1=capAcc[:, :, 2, :]
                )
                # n[b,k] = |capRaw[b,k,:]|^2
                for k in range(K):
                    u = scr.tile([128, D], f32, name="u", tag="u")
                    nc.vector.scalar_tensor_tensor(
                        out=u[:],
                        in0=capRaw[:, k, :],
                        scalar=1.0,
                        in1=capRaw[:, k, :],
                        op0=OP.mult,
                        op1=OP.mult,
                        accum_out=nvec[:, k : k + 1],
                    )
                # s = n / (1+n) / sqrt(n + 1e-9); sqrt via exp(0.5*ln(x)) to
                # stay inside the ln/exp activation table set
                nc.scalar.activation(out=lnt[:], in_=nvec[:], func=AF.Ln, bias=epsB[:])
                nc.scalar.activation(out=rt[:], in_=lnt[:], func=AF.Exp, scale=0.5)
                nc.vector.tensor_scalar_add(out=np1[:], in0=nvec[:], scalar1=1.0)
                bridge(rt[:, 0:1])
                nc.vector.tensor_mul(out=den[:], in0=np1[:], in1=rt[:])
                nc.vector.reciprocal(out=dinv[:], in_=den[:])
                nc.vector.tensor_mul(out=svec[:], in0=nvec[:], in1=dinv[:])

            def cw_update():
                # cw[:, :, k] += s[:, k] * deltaB[:, :, k]
                for k in range(K):
                    nc.vector.scalar_tensor_tensor(
                        out=cw[:, :, k],
                        in0=deltaB[:, :, k],
                        scalar=svec[:, k : k + 1],
                        in1=cw[:, :, k],
                        op0=OP.mult,
                        op1=OP.add,
                    )

            # ---- routing iterations ----
            for it in range(3):
                softmax()
                nc.vector.memset(capAcc[:], 0.0)
                stream(capacc_consume)
                combine_and_squash_scalars()
                if it < 2:
                    stream(delta_consume)
                    cw_update()

            # final: out = s * capRaw
            for k in range(K):
                nc.vector.tensor_scalar_mul(
                    out=capOut[:, k * D : (k + 1) * D],
                    in0=capRaw[:, k, :],
                    scalar1=svec[:, k : k + 1],
                )
            nc.sync.dma_start(out=out_d[:], in_=capOut[:])

    nc.finalize()
    return nc


_NC_CACHE = None


def _get_nc():
    global _NC_CACHE
    if _NC_CACHE is None:
        _NC_CACHE = build_nc()
    return _NC_CACHE


def prep_inputs(seq_out, weights, capsule_weight):
    """Host-side layout prep -> list of per-core input maps."""
    seq = np.ascontiguousarray(np.asarray(seq_out, dtype=np.float32))
    W = np.ascontiguousarray(np.asarray(weights, dtype=np.float32))[0]  # [L,E,D]
    cw = np.ascontiguousarray(np.asarray(capsule_weight, dtype=np.float32))

    # seqT[p, m, b] = seq[b, 2m + p//64, p%64]
    seqT = np.ascontiguousarray(
        seq.reshape(B, M, 2, D).transpose(2, 3, 1, 0).reshape(128, M, B)
    )
    # wT[p, m, e] = W[2m + p//64, e, p%64]
    wT = np.ascontiguousarray(
        W.reshape(M, 2, E, D).transpose(1, 3, 0, 2).reshape(128, M, E)
    )
    # cwA[b, l, k] = cw[b, k, l]
    cwA = np.ascontiguousarray(cw.transpose(0, 2, 1))  # [B, L, K]

    in_maps = []
    for c in range(NCORES):
        in_maps.append(
            {
                "seqT": np.ascontiguousarray(seqT[:, :, c * BS : (c + 1) * BS]),
                "wT": wT,
                "cw": np.ascontiguousarray(cwA[c * BS : (c + 1) * BS]),
            }
        )
    return in_maps


def kernel(seq_out, mask, weights, capsule_weight):
    from concourse.bass_utils import run_bass_kernel_spmd

    nc = _get_nc()
    in_maps = prep_inputs(seq_out, weights, capsule_weight)
    res = run_bass_kernel_spmd(nc, in_maps, core_ids=list(range(NCORES)))
    out = np.concatenate(
        [r["out"].reshape(BS, K, D) for r in res.results], axis=0
    )
    return out.astype(np.float32)


if __name__ == "__main__":
    # smoke test with random data (no reference needed)
    rng = np.random.default_rng(0)
    seq_out = rng.standard_normal((B, L, D), dtype=np.float32)
    mask = np.ones((B, L), dtype=np.float32)
    weights = (0.02 * rng.standard_normal((1, L, E, D))).astype(np.float32)
    capsule_weight = rng.standard_normal((B, K, L), dtype=np.float32)
    out = kernel(seq_out, mask, weights, capsule_weight)
    print("out", out.shape, out.dtype, float(np.abs(out).max()))


# revision 47
# speedup vs baseline: 1.1748x; 1.1748x over previous
"""Trainium2 Bass kernel for the CapsuleNetwork routing problem.

Problem (per reference):
  B, L, D, K = 1024, 200, 64, 4 ; E = K*D = 256
  hat[b,l,e] = sum_d seq[b,l,d] * W[l,e,d]          (einsum, PE)
  3 rounds of dynamic routing over interests K (softmax over K per (b,l)),
  cap = squash(w @ hat), cw += hat . cap            (DVE/ACT)
  output cap -> [B, K, D]

Sharding: pure data-parallel over batch across 8 NeuronCores (128 rows each);
weights replicated. All layout transforms (the d'-contraction transposes) are
host-side so the device sees clean burst DMAs.

Restructured routing algebra (validated vs reference to ~3e-7):
  cw layout [B, L, K];   w = exp(cw) / sum_k exp(cw)
  capRaw[b,k,:] = sum_l w[b,l,k] hat[b,l,k,:]
  n = |capRaw|^2 ; s = n/(1+n)/sqrt(n+1e-9)
  cw += s[b,k] * (hat . capRaw)    (squash scale folded into the cw update)
  final out = s * capRaw

Device layout: hat free axis is (d, k) with k innermost so broadcast access
patterns (0-step dims) keep step-1 innermost on every operand, preserving the
DVE 2x bf16 perf mode for the routing streams. Reductions are pairwise
tree-folds (TT adds at 2x for bf16) instead of 1x tensor_reduce.
"""

import os
import sys

import numpy as np

for _p in ("/opt/trn_rl_repo", "/root/.axon_site/_ro/trn_rl_repo"):
    if os.path.isdir(_p) and _p not in sys.path:
        sys.path.insert(0, _p)

B, L, D, K = 1024, 200, 64, 4
E = K * D
NCORES = 8
BS = B // NCORES  # 128 batch rows per core
M = L // 2        # l-pairs: partition p = (l%2)*64 + d'

# --- tuning flags ---
ROUT_EINSUM_F32R = os.environ.get("KERNEL_F32R", "1") == "1"
PSB = int(os.environ.get("KERNEL_PSB", "1"))  # l's per PSUM tile
PB = 16                   # l's per chunk, routing streams (PSB * n)
PBF = 4                   # l's per chunk, final fp32 stream (PSUM-direct)
PSUM_BUFS = int(os.environ.get("KERNEL_PSUMBUFS", "6"))


def build_nc():
    """Build the Bass program for one core (SPMD; all cores run the same NEFF)."""
    import concourse.bass as bass
    import concourse.tile as tile
    from concourse import bacc, mybir

    f32 = mybir.dt.float32
    f32r = mybir.dt.float32r
    bf16 = mybir.dt.bfloat16
    AF = mybir.ActivationFunctionType
    OP = mybir.AluOpType

    nc = bacc.Bacc(trn_type="TRN2", target_bir_lowering=False, debug=False)
    # seqT/wT typed float32r end-to-end (same bits as fp32 in memory); the
    # routing einsums consume them natively at 1 cyc/row, the final exact
    # pass bitcasts back to float32.
    edt = f32r if ROUT_EINSUM_F32R else f32
    seqT_d = nc.dram_tensor("seqT", [128, M, BS], edt, kind="ExternalInput")
    wT_d = nc.dram_tensor("wT", [128, M, E], edt, kind="ExternalInput")
    cw_d = nc.dram_tensor("cw", [BS, L, K], f32, kind="ExternalInput")
    out_d = nc.dram_tensor("out", [BS, E], f32, kind="ExternalOutput")

    with tile.TileContext(nc) as tc:
        with (
            tc.tile_pool(name="consts", bufs=1) as consts,
            tc.tile_pool(name="hatps", bufs=PSUM_BUFS, space="PSUM") as psum,
            tc.tile_pool(name="hats", bufs=2) as hats,
            tc.tile_pool(name="scr", bufs=2) as scr,
        ):
            seqT = consts.tile([128, M, BS], edt, name="seqT_sb")
            wT = consts.tile([128, M, E], edt, name="wT_sb")
            cw = consts.tile([BS, L, K], f32, name="cw_sb")
            w = consts.tile([BS, L, K], f32, name="w_sb")
            zsum = consts.tile([BS, L], f32, name="zsum")
            zinv = consts.tile([BS, L], f32, name="zinv")
            deltaB = consts.tile([BS, L, K], f32, name="deltaB")
            wB = consts.tile([BS, L, K], bf16, name="wB_sb")
            capB = consts.tile([BS, D, K], bf16, name="capB")
            # capRaw free layout: (d, k) to match hat tiles
            capRaw = consts.tile([BS, D, K], f32, name="capRaw")
            capOut = consts.tile([BS, E], f32, name="capOut")
            # small per-(b,k) scalars packed into one tile (col-sliced)
            smalls = consts.tile([BS, 8, K], f32, name="smalls")
            nvec = smalls[:, 0, :]
            lnt = smalls[:, 1, :]
            rt = smalls[:, 2, :]
            np1 = smalls[:, 3, :]
            den = smalls[:, 4, :]
            dinv = smalls[:, 5, :]
            svec = smalls[:, 6, :]
            epsB = consts.tile([BS, 1], f32, name="epsB")
            nc.vector.memset(epsB[:], 1e-9)

            nc.sync.dma_start(out=cw[:], in_=cw_d[:])
            nc.sync.dma_start(out=seqT[:], in_=seqT_d[:])
            nc.sync.dma_start(out=wT[:], in_=wT_d[:])
            with tc.tile_pool(name="dummyp", bufs=1, space="PSUM") as dummyp:
                dps = dummyp.tile([1, 1], f32, name="dps")
                nc.tensor.matmul(
                    dps[:],
                    lhsT=seqT[0:64, 0, 0:1].bitcast(f32),
                    rhs=seqT[0:64, 0, 0:1].bitcast(f32),
                    start=True,
                    stop=True,
                )

            def softmax():
                # w = softmax over k of cw (k innermost, contiguous)
                nc.scalar.activation(out=w[:], in_=cw[:], func=AF.Exp)
                nc.vector.tensor_reduce(
                    out=zsum[:], in_=w[:], axis=mybir.AxisListType.X, op=OP.add
                )
                nc.vector.reciprocal(out=zinv[:], in_=zsum[:])
                for k in range(K):
                    nc.vector.tensor_mul(out=w[:, :, k], in0=w[:, :, k], in1=zinv[:])
                nc.vector.tensor_copy(out=wB[:], in_=w[:])

            LPB = 512 // E  # l's per PSUM bank (2)

            def emit_matmuls(ps, c0, nl, use_f32r):
                for j in range(nl):
                    l = c0 + j
                    par, m = l % 2, l // 2
                    p0 = 64 * par
                    lhsT = seqT[p0 : p0 + 64, m, :]
                    rhs = wT[p0 : p0 + 64, m, :]
                    if ROUT_EINSUM_F32R and not use_f32r:
                        lhsT = lhsT.bitcast(f32)
                        rhs = rhs.bitcast(f32)
                    # start=True clears the whole PSUM bank: only the first
                    # matmul landing in each bank may set it.
                    nc.tensor.matmul(
                        ps[:, j, :],
                        lhsT=lhsT,
                        rhs=rhs,
                        start=(j % LPB == 0),
                        stop=(j % LPB == LPB - 1 or j == nl - 1),
                        skip_group_check=True,
                    )

            HSDT = bf16 if os.environ.get("KERNEL_HS", "bf16") == "bf16" else f32
            NOCONSUME = os.environ.get("KERNEL_NOCONSUME", "0") == "1"

            def stream(consume, use_f32r):
                """Routing stream: einsum -> PSUM [128,PSB,E] tiles -> ACT
                copies into one bf16 SBUF chunk -> consume(c0, hs_bf16)."""
                for c0 in range(0, L, PB):
                    nl = min(PB, L - c0)
                    hs = hats.tile([128, PB, E], HSDT, name="hs", tag="hs")
                    for b0 in range(0, nl, PSB):
                        ps = psum.tile([128, PSB, E], f32, name="ps", tag="ps")
                        emit_matmuls(ps, c0 + b0, PSB, use_f32r)
                        nc.scalar.copy(out=hs[:, b0 : b0 + PSB, :], in_=ps[:])
                    if not NOCONSUME:
                        consume(c0, hs, nl)

            def capacc_consume(c0, hs, nl):
                # u = hs * w-broadcast (bf16 2x), tree-fold l, fp32 add
                u = scr.tile([128, PB, E], bf16, name="u", tag="u")
                win = bass.AP(
                    tensor=wB.tensor,
                    offset=wB.offset + c0 * K,
                    ap=[wB.ap[0], [K, nl], [0, D], [1, K]],
                )
                nc.vector.tensor_tensor(
                    out=u[:, 0:nl, :], in0=hs[:, 0:nl, :], in1=win, op=OP.mult
                )
                width = nl
                while width > 1:
                    h = width // 2
                    nc.vector.tensor_add(
                        out=u[:, 0:h, :], in0=u[:, 0:h, :], in1=u[:, h : 2 * h, :]
                    )
                    width = h
                nc.vector.tensor_add(out=capRaw[:], in0=capRaw[:], in1=u[:, 0, :])

            def delta_consume(c0, hs, nl):
                u = scr.tile([128, PB, E], bf16, name="u", tag="u")
                cin = bass.AP(
                    tensor=capB.tensor,
                    offset=capB.offset,
                    ap=[capB.ap[0], [0, nl], [1, E]],
                )
                nc.vector.tensor_tensor(
                    out=u[:, 0:nl, :], in0=hs[:, 0:nl, :], in1=cin, op=OP.mult
                )
                # fold d (d-major halves of (d,k) are contiguous slabs)
                width = D
                while width > 2:
                    h = width // 2
                    nc.vector.tensor_add(
                        out=u[:, 0:nl, 0 : h * K],
                        in0=u[:, 0:nl, 0 : h * K],
                        in1=u[:, 0:nl, h * K : 2 * h * K],
                    )
                    width = h
                nc.vector.tensor_add(
                    out=deltaB[:, c0 : c0 + nl, :],
                    in0=u[:, 0:nl, 0:K],
                    in1=u[:, 0:nl, K : 2 * K],
                )

            def final_stream():
                """Final cap pass: fp32 einsum, PSUM-direct fp32 mult+folds."""
                for c0 in range(0, L, PBF):
                    ps = psum.tile([128, PBF, E], f32, name="ps", tag="ps")
                    emit_matmuls(ps, c0, PBF, use_f32r=False)
                    u = scr.tile([128, PBF, E], f32, name="uf", tag="u")
                    win = bass.AP(
                        tensor=w.tensor,
                        offset=w.offset + c0 * K,
                        ap=[w.ap[0], [K, PBF], [0, D], [1, K]],
                    )
                    nc.vector.tensor_tensor(out=u[:], in0=ps[:], in1=win, op=OP.mult)
                    width = PBF
                    while width > 1:
                        h = width // 2
                        nc.vector.tensor_add(
                            out=u[:, 0:h, :],
                            in0=u[:, 0:h, :],
                            in1=u[:, h : 2 * h, :],
                        )
                        width = h
                    nc.vector.tensor_add(
                        out=capRaw[:], in0=capRaw[:], in1=u[:, 0, :]
                    )

            def squash_scalars():
                # n[b,k] = sum_d capRaw[b,d,k]^2 via STT accum per k
                for k in range(K):
                    u2 = scr.tile([128, D], f32, name="u2", tag="u2")
                    nc.vector.scalar_tensor_tensor(
                        out=u2[:],
                        in0=capRaw[:, :, k],
                        scalar=1.0,
                        in1=capRaw[:, :, k],
                        op0=OP.mult,
                        op1=OP.mult,
                        accum_out=nvec[:, k : k + 1],
                    )
                # s = n / (1+n) / sqrt(n + 1e-9); sqrt via exp(0.5*ln(x))
                nc.scalar.activation(out=lnt, in_=nvec, func=AF.Ln, bias=epsB[:])
                nc.scalar.activation(out=rt, in_=lnt, func=AF.Exp, scale=0.5)
                nc.vector.tensor_scalar_add(out=np1, in0=nvec, scalar1=1.0)
                nc.vector.tensor_mul(out=den, in0=np1, in1=rt)
                nc.vector.reciprocal(out=dinv, in_=den)
                nc.vector.tensor_mul(out=svec, in0=nvec, in1=dinv)
                nc.vector.tensor_copy(out=capB[:], in_=capRaw[:])

            def cw_update():
                # cw[:, :, k] += s[:, k] * deltaB[:, :, k]
                for k in range(K):
                    nc.vector.scalar_tensor_tensor(
                        out=cw[:, :, k],
                        in0=deltaB[:, :, k],
                        scalar=svec[:, k : k + 1],
                        in1=cw[:, :, k],
                        op0=OP.mult,
                        op1=OP.add,
                    )

            # ================= routing iterations =================
            A2MODE = os.environ.get("KERNEL_A2", "bf16")
            NITER = int(os.environ.get("KERNEL_NITER", "3"))
            SKIP = set(os.environ.get("KERNEL_SKIP", "").split(","))
            nc.vector.memset(capRaw[:], 1.0)
            nc.vector.memset(smalls[:], 0.5)
            for it in range(NITER):
                final = it == NITER - 1
                if "softmax" not in SKIP:
                    softmax()
                else:
                    nc.vector.memset(w[:], 0.25)
                    nc.vector.tensor_copy(out=wB[:], in_=w[:])
                if "stream" not in SKIP:
                    nc.vector.memset(capRaw[:], 0.0)
                    if not final or A2MODE == "bf16":
                        stream(capacc_consume, use_f32r=ROUT_EINSUM_F32R)
                    else:
                        final_stream()
                if "squash" not in SKIP:
                    squash_scalars()
                if not final:
                    if "delta" not in SKIP:
                        stream(delta_consume, use_f32r=ROUT_EINSUM_F32R)
                    if "cwup" not in SKIP:
                        cw_update()

            # final: out[b, (k,d)] = s[b,k] * capRaw[b, d, k]  (emit (k,d) order)
            for k in range(K):
                nc.vector.tensor_scalar_mul(
                    out=capOut[:, k * D : (k + 1) * D],
                    in0=capRaw[:, :, k],
                    scalar1=svec[:, k : k + 1],
                )
            nc.sync.dma_start(out=out_d[:], in_=capOut[:])

    nc.finalize()
    return nc


def build_tiny():
    """Minimal kernel (DMA in + copy + DMA out) for dispatch-overhead baseline."""
    import concourse.tile as tile
    from concourse import bacc, mybir

    f32 = mybir.dt.float32
    nc = bacc.Bacc(trn_type="TRN2", target_bir_lowering=False, debug=False)
    cw_d = nc.dram_tensor("cw", [BS, L, K], f32, kind="ExternalInput")
    out_d = nc.dram_tensor("out", [BS, E], f32, kind="ExternalOutput")
    with tile.TileContext(nc) as tc:
        with tc.tile_pool(name="p", bufs=1) as p:
            t = p.tile([BS, L, K], f32, name="t_sb")
            o = p.tile([BS, E], f32, name="o_sb")
            nc.sync.dma_start(out=t[:], in_=cw_d[:])
            nc.vector.tensor_copy(out=o[:], in_=t[:, 0:64, :])
            nc.sync.dma_start(out=out_d[:], in_=o[:])
    nc.finalize()
    return nc


_NC_CACHE = None


def _get_nc():
    global _NC_CACHE
    if _NC_CACHE is None:
        _NC_CACHE = build_nc()
    return _NC_CACHE


def prep_inputs(seq_out, weights, capsule_weight):
    """Host-side layout prep -> list of per-core input maps."""
    seq = np.ascontiguousarray(np.asarray(seq_out, dtype=np.float32))
    W = np.ascontiguousarray(np.asarray(weights, dtype=np.float32))[0]  # [L,E,D]
    cwf = np.ascontiguousarray(np.asarray(capsule_weight, dtype=np.float32))

    # seqT[p, m, b] = seq[b, 2m + p//64, p%64]
    seqT = np.ascontiguousarray(
        seq.reshape(B, M, 2, D).transpose(2, 3, 1, 0).reshape(128, M, B)
    )
    # wT[p, m, (d,k)] = W[2m + p//64, k*D + d, p%64]   (hat free axis = (d,k))
    wTf = W.reshape(M, 2, K, D, D).transpose(1, 4, 0, 3, 2)  # [par, d', m, d, k]
    wT = np.ascontiguousarray(wTf.reshape(128, M, E))
    # cwA[b, l, k] = cw[b, k, l]
    cwA = np.ascontiguousarray(cwf.transpose(0, 2, 1))  # [B, L, K]

    in_maps = []
    for c in range(NCORES):
        in_maps.append(
            {
                "seqT": np.ascontiguousarray(seqT[:, :, c * BS : (c + 1) * BS]),
                "wT": wT,
                "cw": np.ascontiguousarray(cwA[c * BS : (c + 1) * BS]),
            }
        )
    return in_maps


def gather_out(results):
    """Per-core 'out' [BS, E=(k*D+d)] -> full [B, K, D]."""
    return np.concatenate(
        [r["out"].reshape(BS, K, D) for r in results], axis=0
    ).astype(np.float32)


def kernel(seq_out, mask, weights, capsule_weight):
    from concourse.bass_utils import run_bass_kernel_spmd

    nc = _get_nc()
    in_maps = prep_inputs(seq_out, weights, capsule_weight)
    res = run_bass_kernel_spmd(nc, in_maps, core_ids=list(range(NCORES)))
    return gather_out(res.results)


if __name__ == "__main__":
    rng = np.random.default_rng(0)
    seq_out = rng.standard_normal((B, L, D), dtype=np.float32)
    mask = np.ones((B, L), dtype=np.float32)
    weights = (0.02 * rng.standard_normal((1, L, E, D))).astype(np.float32)
    capsule_weight = rng.standard_normal((B, K, L), dtype=np.float32)
    out = kernel(seq_out, mask, weights, capsule_weight)
    print("out", out.shape, out.dtype, float(np.abs(out).max()))
